# revision 22
# baseline (speedup 1.0000x reference)
"""GAT (3 layers, heads=1) + global-max-pool + MLP head on 8 Trainium2 NeuronCores.

Sharding: 64 graphs -> 8 cores (8 graphs each; batch is sorted so graphs are
contiguous node ranges).  Graph slot j on every core is padded to a common
length GL[j] so all cores run one identical NEFF (SPMD).  Within a graph,
nodes are sorted by descending in-degree (tightens padded-CSR slot grids).

Per layer: each core computes h_ext = [h | h@a_src] rows (bf16, 256B) for its
own nodes, AllGathers the table to every core, then for each work item
(group of 128-node tiles) gathers the neighbor rows of its own edges with
dma_gather.  Indices are int16, so the table is addressed in 32768-row
windows: each node's neighbor list is split by window, and the slot grid has
per-(item, window) padded depth D_c.  Masked stable segment softmax and the
weighted feature sum run on DVE/ACT; the PE transposes each output tile and
applies the next layer's [W | W@a_src | W@a_dst] in a fused tail.

Performance state (2026-08-08): graded default is GAT_SP=0 (harness-graded
24.89 ms).  NTFF tracing is unavailable in this container (antenv.axon_hooks
missing) -- timing.py's burst methodology (submit N executions, block once,
minus the trivial-kernel burst floor ~5.7 ms) is the only reliable proxy;
per-call wall minus a fixed floor is NOT valid (axon pipelines dispatches).
Burst-measured: SP=0 ~27.5 ms incl. residual dispatch overhead.

Packetized-gather experiment (GAT_SP=1: prepare_only + Tile-managed
trigger_dma(count=None), .sem=gsem flow control bounded to GOUT outstanding
calls, explicit DVE wait_ge(gsem) before the first consumer):
- GPKT=512, TBAT=1: numerically correct (3.0e-4) on every WARM execution,
  but the FIRST execution after NEFF load returns garbage (rel err 1) --
  suspected dirty semaphore/ring state at load; the graded harness runs
  exactly one fresh execution, so this path is not shippable as-is.
  Burst ~29.7 ms: no speedup over SP=0 either, so the 50 ns/row
  single_packet=False cost either isn't the bottleneck under overlap or
  packetization doesn't fix it.  Without the DVE wait_ge the results race
  => Tile's prep DMASW gating does NOT cover DMA completion for consumers.
- GPKT=896 + TBAT=8 + gpsimd sem_clear preamble: NRT_EXEC_UNIT_UNRECOVERABLE
  on first execution (suspect 14 KB packets or batched trigger_n).
Next candidates: (1) GAT_AGC=N chunked AllGathers overlapped with the
producing item loop (code in place, env-gated); (2) isolate the SP
first-execution corruption (sem_clear alone, GPKT=512 TBAT=1) if the SP
path is ever to pay off; (3) per-window degree-sorted grids to cut the
~2.5x gather slot padding (host-side only, no device risk).
"""

import os
import sys
import numpy as np

DBG = int(os.environ.get("GAT_DBG", "0"))
MAXITEMS = int(os.environ.get("GAT_MAXITEMS", "9999"))
MAXCH = int(os.environ.get("GAT_MAXCH", "9999"))
NOPRO = int(os.environ.get("GAT_NOPRO", "0"))
NOCC = int(os.environ.get("GAT_NOCC", "0"))

for _p in ("/opt/trn_rl_repo", "/opt/trn_rl_repo/concourse"):
    if _p not in sys.path:
        sys.path.insert(0, _p)

import concourse.bass as bass  # noqa: E402
import concourse.bacc as bacc  # noqa: E402
import concourse.mybir as mybir  # noqa: E402
import concourse.tile as tile  # noqa: E402
from concourse import library_config  # noqa: E402
from concourse.masks import make_identity  # noqa: E402
from concourse.bass_utils import run_bass_kernel_spmd  # noqa: E402

F32 = mybir.dt.float32
BF16 = mybir.dt.bfloat16
I16 = mybir.dt.int16
ALU = mybir.AluOpType
ACTF = mybir.ActivationFunctionType
AX = mybir.AxisListType

NCORES = 8
NGRAPH = 64
CHUNK = 32768          # int16 index reach per dma_gather call
ROW = 128              # bf16 values per h_ext row = 256B
SLOT_BUDGET = 16384    # max gathered slots per work item (SBUF bound)
MAX_TILES = 16
NEG = -1.0e30
GMAX = 4096         # max idxs per dma_gather call (non-packetized path)
GPKT = int(os.environ.get("GAT_GPKT", "896"))
                    # idxs per single-packet gather; must be a multiple of
                    # 128 with GPKT/16 + 1 <= 64 descs/engine-lane (<=896)
GOUT = int(os.environ.get("GAT_GOUT", "0")) or max(
    4, (1024 - 128) // (GPKT // 16 + 1))
                    # max outstanding packetized calls vs the 1024-desc ring
TBAT = int(os.environ.get("GAT_TBAT", "8"))
                    # preps per trigger_dma (amortizes trigger+sem latency)
SP = int(os.environ.get("GAT_SP", "0"))
                    # packetized prepare_only gathers: correct on warm
                    # executions but still failing the first post-load
                    # execution / crashing at GPKT=896+TBAT=8 -- keep the
                    # proven non-packetized path as the graded default
NAGC = int(os.environ.get("GAT_AGC", "0"))
                    # >0: split each layer's AllGather into NAGC chunks
                    # issued as the producing tiles complete (overlap)


def _ap(t, off, dims):
    return bass.AP(t, off, dims)


# ----------------------------------------------------------------------------
# Host-side preprocessing (sharding / layout + static CSR tables)
# ----------------------------------------------------------------------------

def _preprocess(adj, batch):
    N = batch.shape[0]
    gper = NGRAPH // NCORES
    graph_of = batch.astype(np.int64)
    counts = np.bincount(graph_of, minlength=NGRAPH)
    gstarts = np.zeros(NGRAPH + 1, np.int64)
    np.cumsum(counts, out=gstarts[1:])

    src = np.concatenate([adj[0].astype(np.int64), np.arange(N, dtype=np.int64)])
    dst = np.concatenate([adj[1].astype(np.int64), np.arange(N, dtype=np.int64)])
    deg = np.bincount(dst, minlength=N)

    # common padded per-graph-slot lengths
    glens = counts.reshape(NCORES, gper)
    GL = np.maximum(glens.max(axis=0), 1)            # [gper]
    GST = np.zeros(gper + 1, np.int64)
    np.cumsum(GL, out=GST[1:])
    NPADC = int(np.ceil(GST[-1] / 128) * 128)
    NT = NPADC // 128

    # per-graph degree-desc order; old -> new id (new = core*NPADC + col)
    order = np.lexsort((-deg, graph_of))             # old ids, grouped by graph
    new_of_old = np.empty(N, np.int64)
    order_padded = np.full((NCORES, NPADC), -1, np.int64)
    for g in range(NGRAPH):
        c, j = g // gper, g % gper
        olds = order[gstarts[g]:gstarts[g + 1]]
        col0 = GST[j]
        order_padded[c, col0:col0 + len(olds)] = olds
        new_of_old[olds] = c * NPADC + col0 + np.arange(len(olds))

    NTOT = NCORES * NPADC
    NCH = int((NTOT + CHUNK - 1) // CHUNK)

    nsrc = new_of_old[src]
    ndst = new_of_old[dst]
    dst_core = ndst // NPADC
    dst_local = ndst % NPADC
    ch_src = nsrc // CHUNK
    loc_src = (nsrc % CHUNK).astype(np.int32)

    # per-(core, local node, chunk) degree; max over cores
    degc = np.zeros((NCORES, NPADC, NCH), np.int32)
    np.add.at(degc, (dst_core, dst_local, ch_src), 1)
    degc_max = degc.max(axis=0)

    # shared work-item schedule
    tile_dc = degc_max.reshape(NT, 128, NCH).max(axis=1)
    items = []
    t0 = 0
    while t0 < NT:
        T = 1
        dcur = np.maximum(tile_dc[t0], 1)
        while t0 + T < NT and T < MAX_TILES:
            nd = np.maximum(np.maximum(dcur, tile_dc[t0 + T]), 1)
            if (T + 1) * 128 * int(nd.sum()) > SLOT_BUDGET:
                break
            dcur = nd
            T += 1
        items.append((t0, T, dcur.copy()))
        t0 += T

    # table layouts
    idx_cols = []   # per item: per chunk (col_off, ncols, num_idx)
    msk_cols = []   # per item: col_off
    icol = mcol = 0
    for (ts, T, dc) in items:
        S = int(dc.sum())
        msk_cols.append(mcol)
        mcol += T * S
        row = []
        for c in range(NCH):
            ni = 128 * T * int(dc[c])
            row.append((icol, ni // 16, ni))
            icol += ni // 16
        idx_cols.append(row)
    IDXCOLS, MSKCOLS = icol, mcol

    # per-node lookup arrays for vectorized fill
    item_of_tile = np.zeros(NT, np.int64)
    for ii, (ts, T, dc) in enumerate(items):
        item_of_tile[ts:ts + T] = ii
    arr_ts = np.array([it[0] for it in items], np.int64)
    arr_T = np.array([it[1] for it in items], np.int64)
    arr_S = np.array([int(it[2].sum()) for it in items], np.int64)
    arr_offd = np.zeros((len(items), NCH), np.int64)
    arr_cbase = np.zeros((len(items), NCH), np.int64)
    for ii in range(len(items)):
        off = 0
        for c in range(NCH):
            arr_offd[ii, c] = off
            arr_cbase[ii, c] = idx_cols[ii][c][0]
            off += int(items[ii][2][c])
    arr_mbase = np.array(msk_cols, np.int64)

    idx_tabs, msk_tabs, vlds = [], [], []
    for c in range(NCORES):
        m = dst_core == c
        o = np.lexsort((loc_src[m], ch_src[m], dst_local[m]))
        dl = dst_local[m][o]
        ch = ch_src[m][o]
        lo = loc_src[m][o]
        ne = len(dl)
        # rank within (node, chunk)
        if ne:
            keys = dl * NCH + ch
            brk = np.ones(ne, bool)
            brk[1:] = keys[1:] != keys[:-1]
            gid = np.cumsum(brk) - 1
            gst = np.zeros(gid[-1] + 2 if ne else 1, np.int64)
            np.add.at(gst[1:], gid, 1)
            np.cumsum(gst, out=gst)
            rank = np.arange(ne) - gst[gid]
        else:
            rank = np.zeros(0, np.int64)
        til = dl // 128
        p = dl % 128
        ii = item_of_tile[til]
        t = til - arr_ts[ii]
        T = arr_T[ii]
        S = arr_S[ii]
        D = items[0][2]  # placeholder
        # gather idx table
        q = rank * (T * 128) + t * 128 + p
        col = arr_cbase[ii, ch] + q // 16
        rrow = q % 16
        it = np.zeros((16, IDXCOLS), np.int16)
        it[rrow, col] = lo.astype(np.int16)
        idx_tabs.append(np.tile(it, (8, 1)))
        # mask table
        mt = np.full((128, MSKCOLS), NEG, np.float32)
        mcolv = arr_mbase[ii] + t * S + arr_offd[ii, ch] + rank
        mt[p, mcolv] = 0.0
        # pad nodes: unmask slot (chunk0, j=0) so den=1
        vld = np.zeros((128, NT), np.float32)
        padm = order_padded[c] < 0
        for ti in range(NT):
            iii = item_of_tile[ti]
            tt = ti - arr_ts[iii]
            SS = arr_S[iii]
            prow = np.nonzero(padm[ti * 128:(ti + 1) * 128])[0]
            mt[prow, arr_mbase[iii] + tt * SS] = 0.0
            vld[:, ti] = (~padm[ti * 128:(ti + 1) * 128]).astype(np.float32)
        msk_tabs.append(mt)
        vlds.append(vld)

    return dict(
        N=N, gper=gper, NPADC=NPADC, NT=NT, NTOT=NTOT, NCH=NCH,
        order_padded=order_padded, items=items,
        idx_cols=idx_cols, msk_cols=msk_cols,
        IDXCOLS=IDXCOLS, MSKCOLS=MSKCOLS,
        idx_tabs=idx_tabs, msk_tabs=msk_tabs, vlds=vlds,
        GL=[int(v) for v in GL], GST=[int(v) for v in GST],
        roots=gstarts[:NGRAPH].copy(),
    )


# ----------------------------------------------------------------------------
# Device program
# ----------------------------------------------------------------------------

def _build_program(pp, IN, HID):
    NPADC, NT, NTOT, NCH = pp["NPADC"], pp["NT"], pp["NTOT"], pp["NCH"]
    items, idx_cols, msk_cols = pp["items"], pp["idx_cols"], pp["msk_cols"]
    IDXCOLS, MSKCOLS = pp["IDXCOLS"], pp["MSKCOLS"]
    GL, GST, gper = pp["GL"], pp["GST"], pp["gper"]
    GLMAX = int(np.ceil(max(GL) / 128) * 128)

    nc = bacc.Bacc("TRN2", target_bir_lowering=False, debug=False,
                   num_devices=(1 if NOCC else NCORES))
    gsem = nc.alloc_semaphore("gsem")
    gcall = [0]  # packetized-gather call counter (flow control)
    gpend = [0]  # preps awaiting a trigger_dma

    def _gtrig(force=False):
        if gpend[0] and (force or gpend[0] >= TBAT):
            nc.gpsimd.trigger_dma(count=None)
            gpend[0] = 0

    NT_ = pp["NT"]
    agbnd = ([int(np.ceil(NT_ * (k + 1) / NAGC)) for k in range(NAGC)]
             if NAGC else [])
    agcur = [0, 0, 0]

    def _ag_issue(l, upto_tile, NPADC):
        # AllGather the agin[l] tile ranges fully produced below upto_tile
        while agcur[l] < NAGC and agbnd[agcur[l]] <= upto_tile:
            a = (agbnd[agcur[l] - 1] if agcur[l] else 0) * 128
            b = agbnd[agcur[l]] * 128
            nc.gpsimd.collective_compute(
                "AllGather", ALU.bypass,
                replica_groups=[list(range(NCORES))],
                ins=[_ap(agin[l], a * ROW, [(ROW, b - a), (1, ROW)])],
                outs=[_ap(htab[l], a * ROW,
                          [(NPADC * ROW, NCORES), (ROW, b - a), (1, ROW)])],
            )
            agcur[l] += 1

    xT = nc.dram_tensor("xT", [IN, NPADC], F32, kind="ExternalInput")
    xrootT = nc.dram_tensor("xrootT", [IN, gper], F32, kind="ExternalInput")
    idx_t = nc.dram_tensor("idx", [128, IDXCOLS], I16, kind="ExternalInput")
    msk_t = nc.dram_tensor("msk", [128, MSKCOLS], F32, kind="ExternalInput")
    vld_t = nc.dram_tensor("vld", [128, NT], F32, kind="ExternalInput")
    Ws = {}
    for l, di in ((1, IN), (2, HID), (3, HID)):
        Ws[f"W{l}"] = nc.dram_tensor(f"W{l}", [di, HID], F32, kind="ExternalInput")
        Ws[f"as{l}"] = nc.dram_tensor(f"as{l}", [HID, 1], F32, kind="ExternalInput")
        Ws[f"ad{l}"] = nc.dram_tensor(f"ad{l}", [HID, 1], F32, kind="ExternalInput")
        Ws[f"b{l}"] = nc.dram_tensor(f"b{l}", [128, HID], F32, kind="ExternalInput")
    lin0W = nc.dram_tensor("lin0W", [HID, HID], F32, kind="ExternalInput")
    lin0b = nc.dram_tensor("lin0b", [gper, HID], F32, kind="ExternalInput")
    linnW = nc.dram_tensor("linnW", [IN, HID], F32, kind="ExternalInput")
    linnb = nc.dram_tensor("linnb", [gper, HID], F32, kind="ExternalInput")
    lin1W = nc.dram_tensor("lin1W", [2 * HID, 1], F32, kind="ExternalInput")
    lin1b = nc.dram_tensor("lin1b", [gper, 1], F32, kind="ExternalInput")
    ident_in = nc.dram_tensor("ident", [128, 128], F32, kind="ExternalInput")
    out_t = nc.dram_tensor("out", [gper, 1], F32, kind="ExternalOutput")

    agin = [nc.dram_tensor(f"agin{l}", [NPADC, ROW], BF16, kind="Internal")
            for l in range(3)]
    htab = [nc.dram_tensor(f"htab{l}", [NTOT, ROW], BF16, kind="Internal")
            for l in range(3)]
    x4T_d = nc.dram_tensor("x4T", [HID, NPADC], F32, kind="Internal")

    with tile.TileContext(nc) as tc:
        with (
            tc.tile_pool(name="const", bufs=1) as cpool,
            tc.tile_pool(name="gbuf", bufs=2) as gpool,
            tc.tile_pool(name="pbuf", bufs=2) as ppool,
            tc.tile_pool(name="sbuf", bufs=3) as spool,
            tc.tile_pool(name="psum", bufs=2, space="PSUM") as pspool,
            tc.tile_pool(name="psA", bufs=2, space="PSUM") as psA,
        ):
            ident = cpool.tile([128, 128], F32, tag="ident")
            nc.sync.dma_start(ident[:], ident_in[:])
            if SP:
                # A fresh NEFF load leaves sems at whatever the previous
                # program left; the absolute wait targets below assume 0.
                # Clear on Pool, then force every DVE wait after the clear
                # via a real Pool->DVE data dep (tile syncs it correctly).
                nc.gpsimd.sem_clear(gsem)
                zz = cpool.tile([1, 2], F32, tag="zz")
                nc.gpsimd.memset(zz[:, 0:1], 0.0)
                nc.vector.tensor_copy(zz[:, 1:2], zz[:, 0:1])

            # Wcat_l = [W_l | W_l@a_src | W_l@a_dst], plus bias broadcast
            wcat = []
            s_dst_res = []
            for l, di in ((1, IN), (2, HID), (3, HID)):
                w_sb = cpool.tile([di, HID], F32, tag=f"w{l}")
                nc.sync.dma_start(w_sb[:], Ws[f"W{l}"][:])
                wc = cpool.tile([di, HID + 2], F32, tag=f"wc{l}")
                nc.vector.tensor_copy(wc[:, :HID], w_sb[:])
                if NOPRO:
                    nc.vector.memset(wc[:, HID:], 0.01)
                else:
                    ps_wt = psA.tile([HID, 128], F32, space="PSUM", tag="aux", name="ps_wt")
                    nc.tensor.transpose(ps_wt[:, :di], w_sb[:], ident[:di, :di])
                    wt_sb = cpool.tile([HID, 128], F32, tag=f"wt{l}")
                    nc.scalar.copy(wt_sb[:, :di], ps_wt[:, :di])
                    for name, col in ((f"as{l}", HID), (f"ad{l}", HID + 1)):
                        a_sb = cpool.tile([HID, 1], F32, tag=f"t{name}")
                        nc.sync.dma_start(a_sb[:], Ws[name][:])
                        ps_wa = psA.tile([128, 1], F32, space="PSUM", tag="aux", name="ps_wa")
                        nc.tensor.matmul(ps_wa[:di, :], wt_sb[:, :di], a_sb[:])
                        nc.vector.tensor_copy(wc[:, col:col + 1], ps_wa[:di, :])
                wcat.append(wc)
                b_sb = cpool.tile([128, HID], F32, tag=f"bb{l}")
                nc.sync.dma_start(b_sb[:], Ws[f"b{l}"][:])
                Ws[f"bsb{l}"] = b_sb
                s_dst_res.append(cpool.tile([128, NT], F32, tag=f"sdst{l}", name=f"sdst{l}"))

            vld_sb = cpool.tile([128, NT], F32, tag="vld")
            nc.sync.dma_start(vld_sb[:], vld_t[:])

            # phase A, layer 1
            for t in range(NT):
                x_sb = spool.tile([IN, 128], F32, tag="ax")
                nc.sync.dma_start(x_sb[:], xT[:, t * 128:(t + 1) * 128])
                ps_h = psA.tile([128, HID + 2], F32, space="PSUM", tag="ph", name="ps_h")
                nc.tensor.matmul(ps_h[:], x_sb[:], wcat[0][:])
                hx = spool.tile([128, ROW], BF16, tag="hx")
                nc.vector.memset(hx[:, HID + 1:], 0.0)
                nc.scalar.copy(hx[:, :HID + 1], ps_h[:, :HID + 1])
                nc.vector.tensor_copy(s_dst_res[0][:, t:t + 1],
                                      ps_h[:, HID + 1:HID + 2])
                nc.sync.dma_start(agin[0][t * 128:(t + 1) * 128, :], hx[:])
                if NAGC and not NOCC and not DBG:
                    _ag_issue(0, t + 1 - 4, NPADC)

            # 3 GAT layers
            nlayers = 3 if DBG == 0 else 1
            for l in range(nlayers):
                if NOCC:
                    nc.sync.dma_start(htab[l][0:NPADC, :], agin[l][:])
                elif NAGC and not DBG:
                    _ag_issue(l, NT, NPADC)  # flush remaining chunks
                else:
                    nc.gpsimd.collective_compute(
                        "AllGather", ALU.bypass,
                        replica_groups=[list(range(NCORES))],
                        ins=[agin[l][:]], outs=[htab[l][:]],
                    )
                for ii, (ts, T, dc) in enumerate(items):
                    if DBG == 1 or ii >= MAXITEMS:
                        break
                    S = int(dc.sum())
                    G_sb = gpool.tile([128, 128, ROW], BF16, tag="G")
                    goff = G_sb[:].offset
                    offd = 0
                    for chn in range(min(NCH, MAXCH)):
                        D = int(dc[chn])
                        cbase, ncols, ni = idx_cols[ii][chn]
                        rows_c = min(CHUNK, NTOT - chn * CHUNK)
                        ix = spool.tile([128, ncols], I16, tag="ix",
                                        padded_shape=[128, 2048])
                        nc.sync.dma_start(ix[:],
                                          idx_t[:, cbase:cbase + ncols])
                        in_ap = _ap(htab[l], chn * CHUNK * ROW,
                                    [(ROW, rows_c), (1, ROW)])
                        gstep = GPKT if SP else GMAX
                        for off in range(0, ni, gstep):
                            sni = min(gstep, ni - off)
                            out_ap = _ap(
                                G_sb.tensor,
                                goff + (offd * T + off // 128) * ROW,
                                [(128 * ROW, 128), (ROW, sni // 128), (1, ROW)])
                            if SP:
                                # prepare_only keeps the user DMA sem in its
                                # own slot while Tile still gates data
                                # consumers via the prep's DMASW lane
                                if gcall[0] >= GOUT:
                                    nc.gpsimd.wait_ge(
                                        gsem, 16 * (gcall[0] - GOUT + 1))
                                nc.gpsimd.dma_gather(
                                    out_ap, in_ap,
                                    ix[:, off // 16:(off + sni) // 16],
                                    sni, sni, ROW, prepare_only=True,
                                    sem=gsem, single_packet=True)
                                gpend[0] += 1
                                _gtrig()
                                gcall[0] += 1
                            else:
                                nc.gpsimd.dma_gather(
                                    out_ap, in_ap,
                                    ix[:, off // 16:(off + sni) // 16],
                                    sni, sni, ROW, single_packet=False)
                        offd += D
                    if DBG == 2:
                        continue
                    mbase = msk_cols[ii]
                    mk = spool.tile([128, 128], F32, tag="mk")
                    nc.sync.dma_start(mk[:, :T * S],
                                      msk_t[:, mbase:mbase + T * S])
                    if SP:
                        # gsem is the gathers' DMA-completion sem; gate the
                        # first consumer on every call issued so far (Pool
                        # keeps prepping the next item's gathers meanwhile)
                        _gtrig(force=True)
                        nc.vector.wait_ge(gsem, 16 * gcall[0])
                    mtv = _ap(mk.tensor, mk[:].offset,
                              [(128, 128), (S, T), (1, S)])
                    ssv = _ap(G_sb.tensor, goff + HID,
                              [(128 * ROW, 128), (T * ROW, S), (ROW, T)])
                    e_sb = spool.tile([128, 128], F32, tag="e")
                    ev = _ap(e_sb.tensor, e_sb[:].offset,
                             [(128, 128), (1, S), (S, T)])
                    nc.vector.tensor_copy(ev, ssv)
                    et = _ap(e_sb.tensor, e_sb[:].offset,
                             [(128, 128), (S, T), (1, S)])
                    nc.vector.tensor_tensor(et, et, mtv, ALU.add)
                    sdv = _ap(s_dst_res[l].tensor, s_dst_res[l][:].offset + ts,
                              [(NT, 128), (1, T), (0, S)])
                    nc.vector.tensor_tensor(et, et, sdv, ALU.add)
                    e2_sb = spool.tile([128, 128], F32, tag="e2")
                    e2t = _ap(e2_sb.tensor, e2_sb[:].offset,
                              [(128, 128), (S, T), (1, S)])
                    nc.scalar.activation(e2t, et, ACTF.Copy, scale=0.2)
                    nc.vector.tensor_tensor(et, et, e2t, ALU.max)
                    red = spool.tile([128, MAX_TILES, 4], F32, tag="red")
                    nc.vector.tensor_reduce(red[:, :T, 0:1], et, AX.X, ALU.max)
                    mxb = _ap(red.tensor, red[:].offset,
                              [(MAX_TILES * 4, 128), (4, T), (0, S)])
                    nc.vector.tensor_tensor(et, et, mxb, ALU.subtract)
                    nc.scalar.activation(et, et, ACTF.Exp)
                    nc.vector.tensor_reduce(red[:, :T, 1:2], et, AX.X, ALU.add)
                    nc.vector.reciprocal(red[:, :T, 2:3], red[:, :T, 1:2])
                    nb = spool.tile([128, 128], BF16, tag="nb")
                    nbt = _ap(nb.tensor, nb[:].offset,
                              [(128, 128), (S, T), (1, S)])
                    nc.vector.tensor_copy(nbt, et)
                    # P[t][j][f] = G_h * num
                    P_sb = ppool.tile([128, 128, HID], BF16, tag="P")
                    poff = P_sb[:].offset
                    ghv = _ap(G_sb.tensor, goff,
                              [(128 * ROW, 128), (T * ROW, S), (ROW, T), (1, HID)])
                    nbv = _ap(nb.tensor, nb[:].offset,
                              [(128, 128), (1, S), (S, T), (0, HID)])
                    pv = _ap(P_sb.tensor, poff,
                             [(128 * HID, 128), (HID, S), (S * HID, T), (1, HID)])
                    nc.any.tensor_tensor(pv, ghv, nbv, ALU.mult)
                    o_sb = spool.tile([128, MAX_TILES, HID], F32, tag="o")
                    prd = _ap(P_sb.tensor, poff,
                              [(128 * HID, 128), (S * HID, T), (1, HID), (HID, S)])
                    nc.vector.tensor_reduce(o_sb[:, :T, :], prd, AX.X, ALU.add)
                    rdb = _ap(red.tensor, red[:].offset + 2,
                              [(MAX_TILES * 4, 128), (4, T), (0, HID)])
                    nc.vector.tensor_tensor(o_sb[:, :T, :], o_sb[:, :T, :],
                                            rdb, ALU.mult)
                    bb = _ap(Ws[f"bsb{l + 1}" if l < 2 else "bsb3"].tensor,
                             Ws[f"bsb{l + 1}" if l < 2 else "bsb3"][:].offset,
                             [(HID, 128), (0, T), (1, HID)])
                    nc.vector.tensor_tensor(o_sb[:, :T, :], o_sb[:, :T, :],
                                            bb, ALU.add)
                    nc.scalar.activation(o_sb[:, :T, :], o_sb[:, :T, :],
                                         ACTF.Relu)
                    if l == 2:
                        vb = _ap(vld_sb.tensor, vld_sb[:].offset + ts,
                                 [(NT, 128), (1, T), (0, HID)])
                        nc.vector.tensor_tensor(o_sb[:, :T, :], o_sb[:, :T, :],
                                                vb, ALU.mult)
                    if DBG == 3:
                        continue
                    for t in range(T):
                        ps_t = pspool.tile([HID, 128], F32, space="PSUM")
                        nc.tensor.transpose(ps_t[:], o_sb[:, t, :], ident[:])
                        xt_sb = spool.tile([HID, 128], F32, tag="xt")
                        nc.scalar.copy(xt_sb[:], ps_t[:])
                        if l < 2:
                            ps_h = psA.tile([128, HID + 2], F32, space="PSUM", tag="ph", name="ps_h")
                            nc.tensor.matmul(ps_h[:], xt_sb[:], wcat[l + 1][:])
                            hx = spool.tile([128, ROW], BF16, tag="hx")
                            nc.vector.memset(hx[:, HID + 1:], 0.0)
                            nc.scalar.copy(hx[:, :HID + 1], ps_h[:, :HID + 1])
                            nc.vector.tensor_copy(
                                s_dst_res[l + 1][:, ts + t:ts + t + 1],
                                ps_h[:, HID + 1:HID + 2])
                            nc.sync.dma_start(
                                agin[l + 1][(ts + t) * 128:(ts + t + 1) * 128, :],
                                hx[:])
                        else:
                            nc.sync.dma_start(
                                x4T_d[:, (ts + t) * 128:(ts + t + 1) * 128],
                                xt_sb[:])
                    if NAGC and not NOCC and not DBG and l < 2 and ii >= 2:
                        # AllGather next-layer rows two items behind the
                        # producer so the CC's input wait never stalls Pool
                        pi = items[ii - 2]
                        _ag_issue(l + 1, pi[0] + pi[1], NPADC)

            # head
            if DBG:
                o_dbg = cpool.tile([gper, 1], F32, tag="odbg")
                nc.vector.memset(o_dbg[:], 0.5)
                nc.sync.dma_start(out_t[:], o_dbg[:])
            hmaxT = cpool.tile([HID, gper], F32, tag="hmaxT")
            if DBG:
                hmaxT = None
            for g in range(gper if not DBG else 0):
                hg = spool.tile([HID, GLMAX], F32, tag="hg")
                nc.sync.dma_start(hg[:, :GL[g]], x4T_d[:, GST[g]:GST[g] + GL[g]])
                nc.vector.tensor_reduce(hmaxT[:, g:g + 1], hg[:, :GL[g]],
                                        AX.X, ALU.max)
            if not DBG:
                lw_sb = cpool.tile([HID, HID], F32, tag="l0w")
                nc.sync.dma_start(lw_sb[:], lin0W[:])
                ps_g = psA.tile([gper, HID], F32, space="PSUM", tag="aux", name="ps_g")
                nc.tensor.matmul(ps_g[:], hmaxT[:], lw_sb[:])
                b0_sb = cpool.tile([gper, HID], F32, tag="l0b")
                nc.sync.dma_start(b0_sb[:], lin0b[:])
                h0 = cpool.tile([gper, HID], F32, tag="h0")
                nc.vector.tensor_tensor(h0[:], ps_g[:], b0_sb[:], ALU.add)
                nc.scalar.activation(h0[:], h0[:], ACTF.Relu)

                xr_sb = cpool.tile([IN, gper], F32, tag="xr")
                nc.sync.dma_start(xr_sb[:], xrootT[:])
                lnw_sb = cpool.tile([IN, HID], F32, tag="lnw")
                nc.sync.dma_start(lnw_sb[:], linnW[:])
                ps_n = psA.tile([gper, HID], F32, space="PSUM", tag="aux", name="ps_n")
                nc.tensor.matmul(ps_n[:], xr_sb[:], lnw_sb[:])
                bn_sb = cpool.tile([gper, HID], F32, tag="lnb")
                nc.sync.dma_start(bn_sb[:], linnb[:])
                hn = cpool.tile([gper, HID], F32, tag="hn")
                nc.vector.tensor_tensor(hn[:], ps_n[:], bn_sb[:], ALU.add)
                nc.scalar.activation(hn[:], hn[:], ACTF.Relu)

                catT = cpool.tile([2 * HID, gper], F32, tag="catT")
                ps_t0 = psA.tile([HID, gper], F32, space="PSUM", tag="aux", name="ps_t0")
                nc.tensor.transpose(ps_t0[:], h0[:], ident[:gper, :gper])
                nc.scalar.copy(catT[:HID, :], ps_t0[:])
                ps_t1 = psA.tile([HID, gper], F32, space="PSUM", tag="aux", name="ps_t1")
                nc.tensor.transpose(ps_t1[:], hn[:], ident[:gper, :gper])
                nc.scalar.copy(catT[HID:, :], ps_t1[:])

                l1w_sb = cpool.tile([2 * HID, 1], F32, tag="l1w")
                nc.sync.dma_start(l1w_sb[:], lin1W[:])
                ps_o = psA.tile([gper, 1], F32, space="PSUM", tag="aux", name="ps_o")
                nc.tensor.matmul(ps_o[:], catT[:], l1w_sb[:])
                b1_sb = cpool.tile([gper, 1], F32, tag="l1b")
                nc.sync.dma_start(b1_sb[:], lin1b[:])
                o_fin = cpool.tile([gper, 1], F32, tag="ofin")
                nc.scalar.activation(o_fin[:], ps_o[:], ACTF.Sigmoid, bias=b1_sb[:])
                nc.sync.dma_start(out_t[:], o_fin[:])

    nc.compile()
    return nc


# ----------------------------------------------------------------------------
# entry point
# ----------------------------------------------------------------------------

_CACHE = {}
LAST_RESULTS = None
LAST_NC = None
LAST_INMAPS = None


def kernel(x, adj, batch, W1, a_src1, a_dst1, b1, W2, a_src2, a_dst2, b2,
           W3, a_src3, a_dst3, b3, linnews_W, linnews_b, lin0_W, lin0_b,
           lin1_W, lin1_b):
    x = np.asarray(x)
    adj = np.asarray(adj)
    batch = np.asarray(batch)
    N, IN = x.shape
    HID = np.asarray(W1).shape[1]
    gper = NGRAPH // NCORES

    ckey = (N, adj.shape[1], IN, HID,
            hash(adj.tobytes()), hash(batch.tobytes()))
    if ckey in _CACHE:
        pp, nc = _CACHE[ckey]
    else:
        pp = _preprocess(adj, batch)
        nc = _build_program(pp, IN, HID)
        _CACHE.clear()
        _CACHE[ckey] = (pp, nc)

    NPADC = pp["NPADC"]
    order_padded = pp["order_padded"]
    f32 = np.float32
    in_maps = []
    for c in range(NCORES):
        oc = order_padded[c]
        xc = np.zeros((NPADC, IN), f32)
        real = oc >= 0
        xc[real] = np.asarray(x, f32)[oc[real]]
        roots = pp["roots"][c * gper:(c + 1) * gper]
        im = {
            "xT": np.ascontiguousarray(xc.T),
            "xrootT": np.ascontiguousarray(np.asarray(x, f32)[roots].T),
            "idx": pp["idx_tabs"][c],
            "msk": pp["msk_tabs"][c],
            "vld": pp["vlds"][c],
            "W1": np.asarray(W1, f32), "W2": np.asarray(W2, f32),
            "W3": np.asarray(W3, f32),
            "as1": np.asarray(a_src1, f32).reshape(HID, 1),
            "ad1": np.asarray(a_dst1, f32).reshape(HID, 1),
            "as2": np.asarray(a_src2, f32).reshape(HID, 1),
            "ad2": np.asarray(a_dst2, f32).reshape(HID, 1),
            "as3": np.asarray(a_src3, f32).reshape(HID, 1),
            "ad3": np.asarray(a_dst3, f32).reshape(HID, 1),
            "b1": np.tile(np.asarray(b1, f32).reshape(1, HID), (128, 1)),
            "b2": np.tile(np.asarray(b2, f32).reshape(1, HID), (128, 1)),
            "b3": np.tile(np.asarray(b3, f32).reshape(1, HID), (128, 1)),
            "lin0W": np.asarray(lin0_W, f32),
            "lin0b": np.tile(np.asarray(lin0_b, f32).reshape(1, HID), (gper, 1)),
            "linnW": np.asarray(linnews_W, f32),
            "linnb": np.tile(np.asarray(linnews_b, f32).reshape(1, HID),
                             (gper, 1)),
            "lin1W": np.asarray(lin1_W, f32).reshape(2 * HID, 1),
            "lin1b": np.tile(np.asarray(lin1_b, f32).reshape(1, 1), (gper, 1)),
            "ident": np.eye(128, dtype=f32),
        }
        in_maps.append(im)

    kw = {}
    if os.environ.get("GAT_TRACE", "0") == "1":
        kw = dict(trace=True)
    global LAST_RESULTS, LAST_NC, LAST_INMAPS
    LAST_NC, LAST_INMAPS = nc, in_maps
    res = run_bass_kernel_spmd(nc, in_maps, core_ids=list(range(NCORES)), **kw)
    LAST_RESULTS = res
    out = np.concatenate([res.results[c]["out"] for c in range(NCORES)], axis=0)
    return out.astype(np.float32)



# revision 24
# speedup vs baseline: 1.1702x; 1.1702x over previous
"""GAT (3 layers, heads=1) + global-max-pool + MLP head on 8 Trainium2 NeuronCores.

Sharding: 64 graphs -> 8 cores (8 graphs each; batch is sorted so graphs are
contiguous node ranges).  Graph slot j on every core is padded to a common
length GL[j] so all cores run one identical NEFF (SPMD).  Within a graph,
nodes are sorted by descending in-degree (tightens padded-CSR slot grids).

Per layer: each core computes h_ext = [h | h@a_src] rows (bf16, 256B) for its
own nodes, AllGathers the table to every core, then for each work item
(group of 128-node tiles) gathers the neighbor rows of its own edges with
dma_gather.  Indices are int16, so the table is addressed in 32768-row
windows: each node's neighbor list is split by window, and the slot grid has
per-(item, window) padded depth D_c.  Masked stable segment softmax and the
weighted feature sum run on DVE/ACT; the PE transposes each output tile and
applies the next layer's [W | W@a_src | W@a_dst] in a fused tail.

Performance state (2026-08-08): graded default is GAT_SP=0 (harness-graded
24.89 ms).  NTFF tracing is unavailable in this container (antenv.axon_hooks
missing) -- timing.py's burst methodology (submit N executions, block once,
minus the trivial-kernel burst floor ~5.7 ms) is the only reliable proxy;
per-call wall minus a fixed floor is NOT valid (axon pipelines dispatches).
Burst-measured: SP=0 ~27.5 ms incl. residual dispatch overhead.

Packetized-gather experiment (GAT_SP=1: prepare_only + Tile-managed
trigger_dma(count=None), .sem=gsem flow control bounded to GOUT outstanding
calls, explicit DVE wait_ge(gsem) before the first consumer):
- GPKT=512, TBAT=1: numerically correct (3.0e-4) on every WARM execution,
  but the FIRST execution after NEFF load returns garbage (rel err 1) --
  suspected dirty semaphore/ring state at load; the graded harness runs
  exactly one fresh execution, so this path is not shippable as-is.
  Burst ~29.7 ms: no speedup over SP=0 either, so the 50 ns/row
  single_packet=False cost either isn't the bottleneck under overlap or
  packetization doesn't fix it.  Without the DVE wait_ge the results race
  => Tile's prep DMASW gating does NOT cover DMA completion for consumers.
- GPKT=896 + TBAT=8 + gpsimd sem_clear preamble: NRT_EXEC_UNIT_UNRECOVERABLE
  on first execution (suspect 14 KB packets or batched trigger_n).
Next candidates: (1) GAT_AGC=N chunked AllGathers (code in place, env-gated)
-- BLOCKED: the BIR verifier rejects strided collective outputs ("Output
pattern is not contiguous"), so chunking needs a chunk-major htab layout
(chunk k holds [replica][local rows a_k:b_k] contiguously; remap
new_of_old/nsrc in _preprocess accordingly -- NAGC=0 is the 1-chunk special
case of the same formula); (2) isolate the SP first-execution corruption
(sem_clear alone, GPKT=512 TBAT=1) if the SP path is ever to pay off;
(3) per-window degree-sorted grids to cut the ~2.5x gather slot padding
(host-side only, no device risk).
"""

import os
import sys
import numpy as np

DBG = int(os.environ.get("GAT_DBG", "0"))
MAXITEMS = int(os.environ.get("GAT_MAXITEMS", "9999"))
MAXCH = int(os.environ.get("GAT_MAXCH", "9999"))
NOPRO = int(os.environ.get("GAT_NOPRO", "0"))
NOCC = int(os.environ.get("GAT_NOCC", "0"))

for _p in ("/opt/trn_rl_repo", "/opt/trn_rl_repo/concourse"):
    if _p not in sys.path:
        sys.path.insert(0, _p)

import concourse.bass as bass  # noqa: E402
import concourse.bacc as bacc  # noqa: E402
import concourse.mybir as mybir  # noqa: E402
import concourse.tile as tile  # noqa: E402
from concourse import library_config  # noqa: E402
from concourse.masks import make_identity  # noqa: E402
from concourse.bass_utils import run_bass_kernel_spmd  # noqa: E402

F32 = mybir.dt.float32
BF16 = mybir.dt.bfloat16
I16 = mybir.dt.int16
ALU = mybir.AluOpType
ACTF = mybir.ActivationFunctionType
AX = mybir.AxisListType

NCORES = 8
NGRAPH = 64
CHUNK = 32768          # int16 index reach per dma_gather call
ROW = 128              # bf16 values per h_ext row = 256B
SLOT_BUDGET = 16384    # max gathered slots per work item (SBUF bound)
MAX_TILES = 16
NEG = -1.0e30
GMAX = 4096         # max idxs per dma_gather call (non-packetized path)
GPKT = int(os.environ.get("GAT_GPKT", "896"))
                    # idxs per single-packet gather; must be a multiple of
                    # 128 with GPKT/16 + 1 <= 64 descs/engine-lane (<=896)
GOUT = int(os.environ.get("GAT_GOUT", "0")) or max(
    4, (1024 - 128) // (GPKT // 16 + 1))
                    # max outstanding packetized calls vs the 1024-desc ring
TBAT = int(os.environ.get("GAT_TBAT", "8"))
                    # preps per trigger_dma (amortizes trigger+sem latency)
SP = int(os.environ.get("GAT_SP", "0"))
                    # packetized prepare_only gathers: correct on warm
                    # executions but still failing the first post-load
                    # execution / crashing at GPKT=896+TBAT=8 -- keep the
                    # proven non-packetized path as the graded default
NAGC = int(os.environ.get("GAT_AGC", "0"))
                    # >0: split each layer's AllGather into NAGC chunks
                    # issued as the producing tiles complete (overlap)


def _ap(t, off, dims):
    return bass.AP(t, off, dims)


# ----------------------------------------------------------------------------
# Host-side preprocessing (sharding / layout + static CSR tables)
# ----------------------------------------------------------------------------

def _preprocess(adj, batch):
    N = batch.shape[0]
    gper = NGRAPH // NCORES
    graph_of = batch.astype(np.int64)
    counts = np.bincount(graph_of, minlength=NGRAPH)
    gstarts = np.zeros(NGRAPH + 1, np.int64)
    np.cumsum(counts, out=gstarts[1:])

    src = np.concatenate([adj[0].astype(np.int64), np.arange(N, dtype=np.int64)])
    dst = np.concatenate([adj[1].astype(np.int64), np.arange(N, dtype=np.int64)])
    deg = np.bincount(dst, minlength=N)

    # common padded per-graph-slot lengths
    glens = counts.reshape(NCORES, gper)
    GL = np.maximum(glens.max(axis=0), 1)            # [gper]
    GST = np.zeros(gper + 1, np.int64)
    np.cumsum(GL, out=GST[1:])
    NPADC = int(np.ceil(GST[-1] / 128) * 128)
    NT = NPADC // 128

    # per-graph degree-desc order; old -> new id (new = core*NPADC + col)
    order = np.lexsort((-deg, graph_of))             # old ids, grouped by graph
    new_of_old = np.empty(N, np.int64)
    order_padded = np.full((NCORES, NPADC), -1, np.int64)
    for g in range(NGRAPH):
        c, j = g // gper, g % gper
        olds = order[gstarts[g]:gstarts[g + 1]]
        col0 = GST[j]
        order_padded[c, col0:col0 + len(olds)] = olds
        new_of_old[olds] = c * NPADC + col0 + np.arange(len(olds))

    NTOT = NCORES * NPADC
    NCH = int((NTOT + CHUNK - 1) // CHUNK)

    nsrc = new_of_old[src]
    ndst = new_of_old[dst]
    dst_core = ndst // NPADC
    dst_local = ndst % NPADC
    ch_src = nsrc // CHUNK
    loc_src = (nsrc % CHUNK).astype(np.int32)

    # per-(core, local node, chunk) degree; max over cores
    degc = np.zeros((NCORES, NPADC, NCH), np.int32)
    np.add.at(degc, (dst_core, dst_local, ch_src), 1)
    degc_max = degc.max(axis=0)

    # shared work-item schedule
    tile_dc = degc_max.reshape(NT, 128, NCH).max(axis=1)
    items = []
    t0 = 0
    while t0 < NT:
        T = 1
        dcur = np.maximum(tile_dc[t0], 1)
        while t0 + T < NT and T < MAX_TILES:
            nd = np.maximum(np.maximum(dcur, tile_dc[t0 + T]), 1)
            if (T + 1) * 128 * int(nd.sum()) > SLOT_BUDGET:
                break
            dcur = nd
            T += 1
        items.append((t0, T, dcur.copy()))
        t0 += T

    # table layouts
    idx_cols = []   # per item: per chunk (col_off, ncols, num_idx)
    msk_cols = []   # per item: col_off
    icol = mcol = 0
    for (ts, T, dc) in items:
        S = int(dc.sum())
        msk_cols.append(mcol)
        mcol += T * S
        row = []
        for c in range(NCH):
            ni = 128 * T * int(dc[c])
            row.append((icol, ni // 16, ni))
            icol += ni // 16
        idx_cols.append(row)
    IDXCOLS, MSKCOLS = icol, mcol

    # per-node lookup arrays for vectorized fill
    item_of_tile = np.zeros(NT, np.int64)
    for ii, (ts, T, dc) in enumerate(items):
        item_of_tile[ts:ts + T] = ii
    arr_ts = np.array([it[0] for it in items], np.int64)
    arr_T = np.array([it[1] for it in items], np.int64)
    arr_S = np.array([int(it[2].sum()) for it in items], np.int64)
    arr_offd = np.zeros((len(items), NCH), np.int64)
    arr_cbase = np.zeros((len(items), NCH), np.int64)
    for ii in range(len(items)):
        off = 0
        for c in range(NCH):
            arr_offd[ii, c] = off
            arr_cbase[ii, c] = idx_cols[ii][c][0]
            off += int(items[ii][2][c])
    arr_mbase = np.array(msk_cols, np.int64)

    idx_tabs, msk_tabs, vlds = [], [], []
    for c in range(NCORES):
        m = dst_core == c
        o = np.lexsort((loc_src[m], ch_src[m], dst_local[m]))
        dl = dst_local[m][o]
        ch = ch_src[m][o]
        lo = loc_src[m][o]
        ne = len(dl)
        # rank within (node, chunk)
        if ne:
            keys = dl * NCH + ch
            brk = np.ones(ne, bool)
            brk[1:] = keys[1:] != keys[:-1]
            gid = np.cumsum(brk) - 1
            gst = np.zeros(gid[-1] + 2 if ne else 1, np.int64)
            np.add.at(gst[1:], gid, 1)
            np.cumsum(gst, out=gst)
            rank = np.arange(ne) - gst[gid]
        else:
            rank = np.zeros(0, np.int64)
        til = dl // 128
        p = dl % 128
        ii = item_of_tile[til]
        t = til - arr_ts[ii]
        T = arr_T[ii]
        S = arr_S[ii]
        D = items[0][2]  # placeholder
        # gather idx table
        q = rank * (T * 128) + t * 128 + p
        col = arr_cbase[ii, ch] + q // 16
        rrow = q % 16
        it = np.zeros((16, IDXCOLS), np.int16)
        it[rrow, col] = lo.astype(np.int16)
        idx_tabs.append(np.tile(it, (8, 1)))
        # mask table
        mt = np.full((128, MSKCOLS), NEG, np.float32)
        mcolv = arr_mbase[ii] + t * S + arr_offd[ii, ch] + rank
        mt[p, mcolv] = 0.0
        # pad nodes: unmask slot (chunk0, j=0) so den=1
        vld = np.zeros((128, NT), np.float32)
        padm = order_padded[c] < 0
        for ti in range(NT):
            iii = item_of_tile[ti]
            tt = ti - arr_ts[iii]
            SS = arr_S[iii]
            prow = np.nonzero(padm[ti * 128:(ti + 1) * 128])[0]
            mt[prow, arr_mbase[iii] + tt * SS] = 0.0
            vld[:, ti] = (~padm[ti * 128:(ti + 1) * 128]).astype(np.float32)
        msk_tabs.append(mt)
        vlds.append(vld)

    return dict(
        N=N, gper=gper, NPADC=NPADC, NT=NT, NTOT=NTOT, NCH=NCH,
        order_padded=order_padded, items=items,
        idx_cols=idx_cols, msk_cols=msk_cols,
        IDXCOLS=IDXCOLS, MSKCOLS=MSKCOLS,
        idx_tabs=idx_tabs, msk_tabs=msk_tabs, vlds=vlds,
        GL=[int(v) for v in GL], GST=[int(v) for v in GST],
        roots=gstarts[:NGRAPH].copy(),
    )


# ----------------------------------------------------------------------------
# Device program
# ----------------------------------------------------------------------------

def _build_program(pp, IN, HID):
    NPADC, NT, NTOT, NCH = pp["NPADC"], pp["NT"], pp["NTOT"], pp["NCH"]
    items, idx_cols, msk_cols = pp["items"], pp["idx_cols"], pp["msk_cols"]
    IDXCOLS, MSKCOLS = pp["IDXCOLS"], pp["MSKCOLS"]
    GL, GST, gper = pp["GL"], pp["GST"], pp["gper"]
    GLMAX = int(np.ceil(max(GL) / 128) * 128)

    nc = bacc.Bacc("TRN2", target_bir_lowering=False, debug=False,
                   num_devices=(1 if NOCC else NCORES))
    gsem = nc.alloc_semaphore("gsem")
    gcall = [0]  # packetized-gather call counter (flow control)
    gpend = [0]  # preps awaiting a trigger_dma

    def _gtrig(force=False):
        if gpend[0] and (force or gpend[0] >= TBAT):
            nc.gpsimd.trigger_dma(count=None)
            gpend[0] = 0

    NT_ = pp["NT"]
    agbnd = ([int(np.ceil(NT_ * (k + 1) / NAGC)) for k in range(NAGC)]
             if NAGC else [])
    agcur = [0, 0, 0]

    def _ag_issue(l, upto_tile, NPADC):
        # AllGather the agin[l] tile ranges fully produced below upto_tile
        while agcur[l] < NAGC and agbnd[agcur[l]] <= upto_tile:
            a = (agbnd[agcur[l] - 1] if agcur[l] else 0) * 128
            b = agbnd[agcur[l]] * 128
            nc.gpsimd.collective_compute(
                "AllGather", ALU.bypass,
                replica_groups=[list(range(NCORES))],
                ins=[_ap(agin[l], a * ROW, [(ROW, b - a), (1, ROW)])],
                outs=[_ap(htab[l], a * ROW,
                          [(NPADC * ROW, NCORES), (ROW, b - a), (1, ROW)])],
            )
            agcur[l] += 1

    xT = nc.dram_tensor("xT", [IN, NPADC], F32, kind="ExternalInput")
    xrootT = nc.dram_tensor("xrootT", [IN, gper], F32, kind="ExternalInput")
    idx_t = nc.dram_tensor("idx", [128, IDXCOLS], I16, kind="ExternalInput")
    msk_t = nc.dram_tensor("msk", [128, MSKCOLS], F32, kind="ExternalInput")
    vld_t = nc.dram_tensor("vld", [128, NT], F32, kind="ExternalInput")
    Ws = {}
    for l, di in ((1, IN), (2, HID), (3, HID)):
        Ws[f"W{l}"] = nc.dram_tensor(f"W{l}", [di, HID], F32, kind="ExternalInput")
        Ws[f"as{l}"] = nc.dram_tensor(f"as{l}", [HID, 1], F32, kind="ExternalInput")
        Ws[f"ad{l}"] = nc.dram_tensor(f"ad{l}", [HID, 1], F32, kind="ExternalInput")
        Ws[f"b{l}"] = nc.dram_tensor(f"b{l}", [128, HID], F32, kind="ExternalInput")
    lin0W = nc.dram_tensor("lin0W", [HID, HID], F32, kind="ExternalInput")
    lin0b = nc.dram_tensor("lin0b", [gper, HID], F32, kind="ExternalInput")
    linnW = nc.dram_tensor("linnW", [IN, HID], F32, kind="ExternalInput")
    linnb = nc.dram_tensor("linnb", [gper, HID], F32, kind="ExternalInput")
    lin1W = nc.dram_tensor("lin1W", [2 * HID, 1], F32, kind="ExternalInput")
    lin1b = nc.dram_tensor("lin1b", [gper, 1], F32, kind="ExternalInput")
    ident_in = nc.dram_tensor("ident", [128, 128], F32, kind="ExternalInput")
    out_t = nc.dram_tensor("out", [gper, 1], F32, kind="ExternalOutput")

    agin = [nc.dram_tensor(f"agin{l}", [NPADC, ROW], BF16, kind="Internal")
            for l in range(3)]
    # Shared addr space: HBM-HBM AllGather outputs take the fast path
    # (the compiler warns on Local); Internal+Shared is the supported combo
    _hsp = "Shared" if int(os.environ.get("GAT_SHARED", "1")) else "Local"
    htab = [nc.dram_tensor(f"htab{l}", [NTOT, ROW], BF16, kind="Internal",
                           addr_space=_hsp)
            for l in range(3)]
    x4T_d = nc.dram_tensor("x4T", [HID, NPADC], F32, kind="Internal")

    with tile.TileContext(nc) as tc:
        with (
            tc.tile_pool(name="const", bufs=1) as cpool,
            tc.tile_pool(name="gbuf", bufs=2) as gpool,
            tc.tile_pool(name="pbuf", bufs=2) as ppool,
            tc.tile_pool(name="sbuf", bufs=3) as spool,
            tc.tile_pool(name="psum", bufs=2, space="PSUM") as pspool,
            tc.tile_pool(name="psA", bufs=2, space="PSUM") as psA,
        ):
            ident = cpool.tile([128, 128], F32, tag="ident")
            nc.sync.dma_start(ident[:], ident_in[:])
            if SP:
                # A fresh NEFF load leaves sems at whatever the previous
                # program left; the absolute wait targets below assume 0.
                # Clear on Pool, then force every DVE wait after the clear
                # via a real Pool->DVE data dep (tile syncs it correctly).
                nc.gpsimd.sem_clear(gsem)
                zz = cpool.tile([1, 2], F32, tag="zz")
                nc.gpsimd.memset(zz[:, 0:1], 0.0)
                nc.vector.tensor_copy(zz[:, 1:2], zz[:, 0:1])

            # Wcat_l = [W_l | W_l@a_src | W_l@a_dst], plus bias broadcast
            wcat = []
            s_dst_res = []
            for l, di in ((1, IN), (2, HID), (3, HID)):
                w_sb = cpool.tile([di, HID], F32, tag=f"w{l}")
                nc.sync.dma_start(w_sb[:], Ws[f"W{l}"][:])
                wc = cpool.tile([di, HID + 2], F32, tag=f"wc{l}")
                nc.vector.tensor_copy(wc[:, :HID], w_sb[:])
                if NOPRO:
                    nc.vector.memset(wc[:, HID:], 0.01)
                else:
                    ps_wt = psA.tile([HID, 128], F32, space="PSUM", tag="aux", name="ps_wt")
                    nc.tensor.transpose(ps_wt[:, :di], w_sb[:], ident[:di, :di])
                    wt_sb = cpool.tile([HID, 128], F32, tag=f"wt{l}")
                    nc.scalar.copy(wt_sb[:, :di], ps_wt[:, :di])
                    for name, col in ((f"as{l}", HID), (f"ad{l}", HID + 1)):
                        a_sb = cpool.tile([HID, 1], F32, tag=f"t{name}")
                        nc.sync.dma_start(a_sb[:], Ws[name][:])
                        ps_wa = psA.tile([128, 1], F32, space="PSUM", tag="aux", name="ps_wa")
                        nc.tensor.matmul(ps_wa[:di, :], wt_sb[:, :di], a_sb[:])
                        nc.vector.tensor_copy(wc[:, col:col + 1], ps_wa[:di, :])
                wcat.append(wc)
                b_sb = cpool.tile([128, HID], F32, tag=f"bb{l}")
                nc.sync.dma_start(b_sb[:], Ws[f"b{l}"][:])
                Ws[f"bsb{l}"] = b_sb
                s_dst_res.append(cpool.tile([128, NT], F32, tag=f"sdst{l}", name=f"sdst{l}"))

            vld_sb = cpool.tile([128, NT], F32, tag="vld")
            nc.sync.dma_start(vld_sb[:], vld_t[:])

            # phase A, layer 1
            for t in range(NT):
                x_sb = spool.tile([IN, 128], F32, tag="ax")
                nc.sync.dma_start(x_sb[:], xT[:, t * 128:(t + 1) * 128])
                ps_h = psA.tile([128, HID + 2], F32, space="PSUM", tag="ph", name="ps_h")
                nc.tensor.matmul(ps_h[:], x_sb[:], wcat[0][:])
                hx = spool.tile([128, ROW], BF16, tag="hx")
                nc.vector.memset(hx[:, HID + 1:], 0.0)
                nc.scalar.copy(hx[:, :HID + 1], ps_h[:, :HID + 1])
                nc.vector.tensor_copy(s_dst_res[0][:, t:t + 1],
                                      ps_h[:, HID + 1:HID + 2])
                nc.sync.dma_start(agin[0][t * 128:(t + 1) * 128, :], hx[:])
                if NAGC and not NOCC and not DBG:
                    _ag_issue(0, t + 1 - 4, NPADC)

            # 3 GAT layers
            nlayers = 3 if DBG == 0 else 1
            for l in range(nlayers):
                if NOCC:
                    nc.sync.dma_start(htab[l][0:NPADC, :], agin[l][:])
                elif NAGC and not DBG:
                    _ag_issue(l, NT, NPADC)  # flush remaining chunks
                else:
                    nc.gpsimd.collective_compute(
                        "AllGather", ALU.bypass,
                        replica_groups=[list(range(NCORES))],
                        ins=[agin[l][:]], outs=[htab[l][:]],
                    )
                for ii, (ts, T, dc) in enumerate(items):
                    if DBG == 1 or ii >= MAXITEMS:
                        break
                    S = int(dc.sum())
                    G_sb = gpool.tile([128, 128, ROW], BF16, tag="G")
                    goff = G_sb[:].offset
                    offd = 0
                    for chn in range(min(NCH, MAXCH)):
                        D = int(dc[chn])
                        cbase, ncols, ni = idx_cols[ii][chn]
                        rows_c = min(CHUNK, NTOT - chn * CHUNK)
                        ix = spool.tile([128, ncols], I16, tag="ix",
                                        padded_shape=[128, 2048])
                        nc.sync.dma_start(ix[:],
                                          idx_t[:, cbase:cbase + ncols])
                        in_ap = _ap(htab[l], chn * CHUNK * ROW,
                                    [(ROW, rows_c), (1, ROW)])
                        gstep = GPKT if SP else GMAX
                        for off in range(0, ni, gstep):
                            sni = min(gstep, ni - off)
                            out_ap = _ap(
                                G_sb.tensor,
                                goff + (offd * T + off // 128) * ROW,
                                [(128 * ROW, 128), (ROW, sni // 128), (1, ROW)])
                            if SP:
                                # prepare_only keeps the user DMA sem in its
                                # own slot while Tile still gates data
                                # consumers via the prep's DMASW lane
                                if gcall[0] >= GOUT:
                                    nc.gpsimd.wait_ge(
                                        gsem, 16 * (gcall[0] - GOUT + 1))
                                nc.gpsimd.dma_gather(
                                    out_ap, in_ap,
                                    ix[:, off // 16:(off + sni) // 16],
                                    sni, sni, ROW, prepare_only=True,
                                    sem=gsem, single_packet=True)
                                gpend[0] += 1
                                _gtrig()
                                gcall[0] += 1
                            else:
                                nc.gpsimd.dma_gather(
                                    out_ap, in_ap,
                                    ix[:, off // 16:(off + sni) // 16],
                                    sni, sni, ROW, single_packet=False)
                        offd += D
                    if DBG == 2:
                        continue
                    mbase = msk_cols[ii]
                    mk = spool.tile([128, 128], F32, tag="mk")
                    nc.sync.dma_start(mk[:, :T * S],
                                      msk_t[:, mbase:mbase + T * S])
                    if SP:
                        # gsem is the gathers' DMA-completion sem; gate the
                        # first consumer on every call issued so far (Pool
                        # keeps prepping the next item's gathers meanwhile)
                        _gtrig(force=True)
                        nc.vector.wait_ge(gsem, 16 * gcall[0])
                    mtv = _ap(mk.tensor, mk[:].offset,
                              [(128, 128), (S, T), (1, S)])
                    ssv = _ap(G_sb.tensor, goff + HID,
                              [(128 * ROW, 128), (T * ROW, S), (ROW, T)])
                    e_sb = spool.tile([128, 128], F32, tag="e")
                    ev = _ap(e_sb.tensor, e_sb[:].offset,
                             [(128, 128), (1, S), (S, T)])
                    nc.vector.tensor_copy(ev, ssv)
                    et = _ap(e_sb.tensor, e_sb[:].offset,
                             [(128, 128), (S, T), (1, S)])
                    nc.vector.tensor_tensor(et, et, mtv, ALU.add)
                    sdv = _ap(s_dst_res[l].tensor, s_dst_res[l][:].offset + ts,
                              [(NT, 128), (1, T), (0, S)])
                    nc.vector.tensor_tensor(et, et, sdv, ALU.add)
                    e2_sb = spool.tile([128, 128], F32, tag="e2")
                    e2t = _ap(e2_sb.tensor, e2_sb[:].offset,
                              [(128, 128), (S, T), (1, S)])
                    nc.scalar.activation(e2t, et, ACTF.Copy, scale=0.2)
                    nc.vector.tensor_tensor(et, et, e2t, ALU.max)
                    red = spool.tile([128, MAX_TILES, 4], F32, tag="red")
                    nc.vector.tensor_reduce(red[:, :T, 0:1], et, AX.X, ALU.max)
                    mxb = _ap(red.tensor, red[:].offset,
                              [(MAX_TILES * 4, 128), (4, T), (0, S)])
                    nc.vector.tensor_tensor(et, et, mxb, ALU.subtract)
                    nc.scalar.activation(et, et, ACTF.Exp)
                    nc.vector.tensor_reduce(red[:, :T, 1:2], et, AX.X, ALU.add)
                    nc.vector.reciprocal(red[:, :T, 2:3], red[:, :T, 1:2])
                    nb = spool.tile([128, 128], BF16, tag="nb")
                    nbt = _ap(nb.tensor, nb[:].offset,
                              [(128, 128), (S, T), (1, S)])
                    nc.vector.tensor_copy(nbt, et)
                    # P[t][j][f] = G_h * num
                    P_sb = ppool.tile([128, 128, HID], BF16, tag="P")
                    poff = P_sb[:].offset
                    ghv = _ap(G_sb.tensor, goff,
                              [(128 * ROW, 128), (T * ROW, S), (ROW, T), (1, HID)])
                    nbv = _ap(nb.tensor, nb[:].offset,
                              [(128, 128), (1, S), (S, T), (0, HID)])
                    pv = _ap(P_sb.tensor, poff,
                             [(128 * HID, 128), (HID, S), (S * HID, T), (1, HID)])
                    nc.any.tensor_tensor(pv, ghv, nbv, ALU.mult)
                    o_sb = spool.tile([128, MAX_TILES, HID], F32, tag="o")
                    prd = _ap(P_sb.tensor, poff,
                              [(128 * HID, 128), (S * HID, T), (1, HID), (HID, S)])
                    nc.vector.tensor_reduce(o_sb[:, :T, :], prd, AX.X, ALU.add)
                    rdb = _ap(red.tensor, red[:].offset + 2,
                              [(MAX_TILES * 4, 128), (4, T), (0, HID)])
                    nc.vector.tensor_tensor(o_sb[:, :T, :], o_sb[:, :T, :],
                                            rdb, ALU.mult)
                    bb = _ap(Ws[f"bsb{l + 1}" if l < 2 else "bsb3"].tensor,
                             Ws[f"bsb{l + 1}" if l < 2 else "bsb3"][:].offset,
                             [(HID, 128), (0, T), (1, HID)])
                    nc.vector.tensor_tensor(o_sb[:, :T, :], o_sb[:, :T, :],
                                            bb, ALU.add)
                    nc.scalar.activation(o_sb[:, :T, :], o_sb[:, :T, :],
                                         ACTF.Relu)
                    if l == 2:
                        vb = _ap(vld_sb.tensor, vld_sb[:].offset + ts,
                                 [(NT, 128), (1, T), (0, HID)])
                        nc.vector.tensor_tensor(o_sb[:, :T, :], o_sb[:, :T, :],
                                                vb, ALU.mult)
                    if DBG == 3:
                        continue
                    for t in range(T):
                        ps_t = pspool.tile([HID, 128], F32, space="PSUM")
                        nc.tensor.transpose(ps_t[:], o_sb[:, t, :], ident[:])
                        xt_sb = spool.tile([HID, 128], F32, tag="xt")
                        nc.scalar.copy(xt_sb[:], ps_t[:])
                        if l < 2:
                            ps_h = psA.tile([128, HID + 2], F32, space="PSUM", tag="ph", name="ps_h")
                            nc.tensor.matmul(ps_h[:], xt_sb[:], wcat[l + 1][:])
                            hx = spool.tile([128, ROW], BF16, tag="hx")
                            nc.vector.memset(hx[:, HID + 1:], 0.0)
                            nc.scalar.copy(hx[:, :HID + 1], ps_h[:, :HID + 1])
                            nc.vector.tensor_copy(
                                s_dst_res[l + 1][:, ts + t:ts + t + 1],
                                ps_h[:, HID + 1:HID + 2])
                            nc.sync.dma_start(
                                agin[l + 1][(ts + t) * 128:(ts + t + 1) * 128, :],
                                hx[:])
                        else:
                            nc.sync.dma_start(
                                x4T_d[:, (ts + t) * 128:(ts + t + 1) * 128],
                                xt_sb[:])
                    if NAGC and not NOCC and not DBG and l < 2 and ii >= 2:
                        # AllGather next-layer rows two items behind the
                        # producer so the CC's input wait never stalls Pool
                        pi = items[ii - 2]
                        _ag_issue(l + 1, pi[0] + pi[1], NPADC)

            # head
            if DBG:
                o_dbg = cpool.tile([gper, 1], F32, tag="odbg")
                nc.vector.memset(o_dbg[:], 0.5)
                nc.sync.dma_start(out_t[:], o_dbg[:])
            hmaxT = cpool.tile([HID, gper], F32, tag="hmaxT")
            if DBG:
                hmaxT = None
            for g in range(gper if not DBG else 0):
                hg = spool.tile([HID, GLMAX], F32, tag="hg")
                nc.sync.dma_start(hg[:, :GL[g]], x4T_d[:, GST[g]:GST[g] + GL[g]])
                nc.vector.tensor_reduce(hmaxT[:, g:g + 1], hg[:, :GL[g]],
                                        AX.X, ALU.max)
            if not DBG:
                lw_sb = cpool.tile([HID, HID], F32, tag="l0w")
                nc.sync.dma_start(lw_sb[:], lin0W[:])
                ps_g = psA.tile([gper, HID], F32, space="PSUM", tag="aux", name="ps_g")
                nc.tensor.matmul(ps_g[:], hmaxT[:], lw_sb[:])
                b0_sb = cpool.tile([gper, HID], F32, tag="l0b")
                nc.sync.dma_start(b0_sb[:], lin0b[:])
                h0 = cpool.tile([gper, HID], F32, tag="h0")
                nc.vector.tensor_tensor(h0[:], ps_g[:], b0_sb[:], ALU.add)
                nc.scalar.activation(h0[:], h0[:], ACTF.Relu)

                xr_sb = cpool.tile([IN, gper], F32, tag="xr")
                nc.sync.dma_start(xr_sb[:], xrootT[:])
                lnw_sb = cpool.tile([IN, HID], F32, tag="lnw")
                nc.sync.dma_start(lnw_sb[:], linnW[:])
                ps_n = psA.tile([gper, HID], F32, space="PSUM", tag="aux", name="ps_n")
                nc.tensor.matmul(ps_n[:], xr_sb[:], lnw_sb[:])
                bn_sb = cpool.tile([gper, HID], F32, tag="lnb")
                nc.sync.dma_start(bn_sb[:], linnb[:])
                hn = cpool.tile([gper, HID], F32, tag="hn")
                nc.vector.tensor_tensor(hn[:], ps_n[:], bn_sb[:], ALU.add)
                nc.scalar.activation(hn[:], hn[:], ACTF.Relu)

                catT = cpool.tile([2 * HID, gper], F32, tag="catT")
                ps_t0 = psA.tile([HID, gper], F32, space="PSUM", tag="aux", name="ps_t0")
                nc.tensor.transpose(ps_t0[:], h0[:], ident[:gper, :gper])
                nc.scalar.copy(catT[:HID, :], ps_t0[:])
                ps_t1 = psA.tile([HID, gper], F32, space="PSUM", tag="aux", name="ps_t1")
                nc.tensor.transpose(ps_t1[:], hn[:], ident[:gper, :gper])
                nc.scalar.copy(catT[HID:, :], ps_t1[:])

                l1w_sb = cpool.tile([2 * HID, 1], F32, tag="l1w")
                nc.sync.dma_start(l1w_sb[:], lin1W[:])
                ps_o = psA.tile([gper, 1], F32, space="PSUM", tag="aux", name="ps_o")
                nc.tensor.matmul(ps_o[:], catT[:], l1w_sb[:])
                b1_sb = cpool.tile([gper, 1], F32, tag="l1b")
                nc.sync.dma_start(b1_sb[:], lin1b[:])
                o_fin = cpool.tile([gper, 1], F32, tag="ofin")
                nc.scalar.activation(o_fin[:], ps_o[:], ACTF.Sigmoid, bias=b1_sb[:])
                nc.sync.dma_start(out_t[:], o_fin[:])

    nc.compile()
    return nc


# ----------------------------------------------------------------------------
# entry point
# ----------------------------------------------------------------------------

_CACHE = {}
LAST_RESULTS = None
LAST_NC = None
LAST_INMAPS = None


def kernel(x, adj, batch, W1, a_src1, a_dst1, b1, W2, a_src2, a_dst2, b2,
           W3, a_src3, a_dst3, b3, linnews_W, linnews_b, lin0_W, lin0_b,
           lin1_W, lin1_b):
    x = np.asarray(x)
    adj = np.asarray(adj)
    batch = np.asarray(batch)
    N, IN = x.shape
    HID = np.asarray(W1).shape[1]
    gper = NGRAPH // NCORES

    ckey = (N, adj.shape[1], IN, HID,
            hash(adj.tobytes()), hash(batch.tobytes()))
    if ckey in _CACHE:
        pp, nc = _CACHE[ckey]
    else:
        pp = _preprocess(adj, batch)
        nc = _build_program(pp, IN, HID)
        _CACHE.clear()
        _CACHE[ckey] = (pp, nc)

    NPADC = pp["NPADC"]
    order_padded = pp["order_padded"]
    f32 = np.float32
    in_maps = []
    for c in range(NCORES):
        oc = order_padded[c]
        xc = np.zeros((NPADC, IN), f32)
        real = oc >= 0
        xc[real] = np.asarray(x, f32)[oc[real]]
        roots = pp["roots"][c * gper:(c + 1) * gper]
        im = {
            "xT": np.ascontiguousarray(xc.T),
            "xrootT": np.ascontiguousarray(np.asarray(x, f32)[roots].T),
            "idx": pp["idx_tabs"][c],
            "msk": pp["msk_tabs"][c],
            "vld": pp["vlds"][c],
            "W1": np.asarray(W1, f32), "W2": np.asarray(W2, f32),
            "W3": np.asarray(W3, f32),
            "as1": np.asarray(a_src1, f32).reshape(HID, 1),
            "ad1": np.asarray(a_dst1, f32).reshape(HID, 1),
            "as2": np.asarray(a_src2, f32).reshape(HID, 1),
            "ad2": np.asarray(a_dst2, f32).reshape(HID, 1),
            "as3": np.asarray(a_src3, f32).reshape(HID, 1),
            "ad3": np.asarray(a_dst3, f32).reshape(HID, 1),
            "b1": np.tile(np.asarray(b1, f32).reshape(1, HID), (128, 1)),
            "b2": np.tile(np.asarray(b2, f32).reshape(1, HID), (128, 1)),
            "b3": np.tile(np.asarray(b3, f32).reshape(1, HID), (128, 1)),
            "lin0W": np.asarray(lin0_W, f32),
            "lin0b": np.tile(np.asarray(lin0_b, f32).reshape(1, HID), (gper, 1)),
            "linnW": np.asarray(linnews_W, f32),
            "linnb": np.tile(np.asarray(linnews_b, f32).reshape(1, HID),
                             (gper, 1)),
            "lin1W": np.asarray(lin1_W, f32).reshape(2 * HID, 1),
            "lin1b": np.tile(np.asarray(lin1_b, f32).reshape(1, 1), (gper, 1)),
            "ident": np.eye(128, dtype=f32),
        }
        in_maps.append(im)

    kw = {}
    if os.environ.get("GAT_TRACE", "0") == "1":
        kw = dict(trace=True)
    global LAST_RESULTS, LAST_NC, LAST_INMAPS
    LAST_NC, LAST_INMAPS = nc, in_maps
    res = run_bass_kernel_spmd(nc, in_maps, core_ids=list(range(NCORES)), **kw)
    LAST_RESULTS = res
    out = np.concatenate([res.results[c]["out"] for c in range(NCORES)], axis=0)
    return out.astype(np.float32)

